# revision 11
# baseline (speedup 1.0000x reference)
"""Trainium2 Bass kernel for nn_AE_gnnrnn (biLSTM encoder -> GCN fusion ->
single-step biLSTM decoder -> vocab projection), SPMD across 8 NeuronCores.

Sharding: data-parallel over nodes N=128 -> 16 nodes/core. Weights replicated.
The only cross-core exchange is an AllGather of the [26,16] per-core encoder
states (the GCN needs all nodes); the GCN itself is tiny and replicated.

Schedule insight: decoder timesteps l>=1 depend ONLY on x_tokens (the
reference feeds the GNN state at step 0 and zeros elsewhere), so the dominant
[2048,27]x[27,8000] output projection + 65MB/core DMA-out runs concurrently
with the serial LSTM scan + collective + GCN, which gate only the 16 l=0
output rows (M-chunk 0, emitted last).

Hardware layout constraint: compute-engine partition ranges must start at a
quadrant boundary (0/32/64/96), so LSTM gates are padded to quadrants
(i@0, f@32, gg@64, o@96) and the decoder feature dim to [f@0, b@32, bias@64].
"""

import numpy as np

import concourse.bass as bass
import concourse.mybir as mybir
import concourse.tile as tile
from concourse.bass import AP, IndirectOffsetOnAxis
from concourse.bass_utils import run_bass_kernel_spmd
from concourse.masks import make_identity
from concourse.vector_clock import ScopedClock, VectorClock

F32 = mybir.dt.float32
BF16 = mybir.dt.bfloat16
I32 = mybir.dt.int32
AF = mybir.ActivationFunctionType
ALU = mybir.AluOpType

N_CORES = 8
N, L, V, IN_DIM, H, E = 128, 128, 8000, 64, 13, 2048
NL = 16              # nodes per core
D2 = 2 * H           # 26
ROWS = NL * L        # 2048; l-major: row = l*16 + n
NCH = 17             # edge chunks of 128 (16 real + 1 self-loop)
VC = 500             # vocab chunk (16 x 500 = 8000)
NVC = V // VC
NMC = ROWS // 128    # 16 M-chunks; chunk m covers l in [8m, 8m+8)
GP = 128             # padded gate dim (i@0, f@32, gg@64, o@96)
Q = 32
DR = 65              # decoder feature rows: f@0:13, b@32:45, bias@64

_PATCHED = False


def split_multi_waits(bir_bytes):
    """This container's walrus accepts at most ONE sync wait per instruction.
    Tile attaches several. Hoist extra waits onto single-wait EventSemaphore
    carriers inserted immediately before the owning instruction (same
    engine/queue), which is semantically identical: the engine blocks on each
    in program order."""
    import json
    bir = json.loads(bir_bytes)
    ctr = 0
    for fn in bir["functions"]:
        for blk in fn["blocks"]:
            new_list = []
            for ins in blk["instructions"]:
                si = ins.get("sync_info")
                waits = (si or {}).get("on_wait") or []
                if len(waits) > 1:
                    for w in waits[:-1]:
                        ctr += 1
                        carrier = {
                            "name": f"evw-{ctr}",
                            "opcode": "EventSemaphore",
                            "engine": ins.get("engine"),
                            "ins": [],
                            "outs": [],
                            "sync_info": {"on_wait": [w], "on_update": []},
                        }
                        if "debug" in ins:
                            carrier["debug"] = ins["debug"]
                        if "queue" in ins:
                            carrier["queue"] = ins["queue"]
                        new_list.append(carrier)
                    si["on_wait"] = [waits[-1]]
                new_list.append(ins)
            blk["instructions"] = new_list
    return json.dumps(bir).encode()


def _patch_tail_drain():
    """Workarounds for this container's walrus wait-slot limit."""
    global _PATCHED
    if _PATCHED:
        return
    _PATCHED = True

    def _patched(self, tick_clock, wait_clock):
        nc = self.nc
        gc = tick_clock.global_clock
        for p in range(len(gc)):
            t = gc[p]
            if t > 0:
                vc = VectorClock()
                vc.require_at_least(p, t)
                nop = nc.sync.nop(nofuse=True, hint=f"tail_wait_p{p}")
                wait_clock.add_sem_waits(nop.ins, ScopedClock({None: vc}))
        nc.sync.drain()
        nc.all_engine_barrier()
        popped = nc._tile_sem_poison_stack.pop()
        assert popped is self._sem_poison
        nc.clear_and_free_semaphores(list(self.sems.allocated().values()))
        nc.all_engine_barrier()

    tile.TileContext._drain_and_barrier = _patched

    # route every BIR compile through the multi-wait splitter
    from concourse import bass_utils as _bu
    from concourse import bass2jax as _b2j
    _orig_compile = _bu.compile_bir_kernel

    def _compile_hook(bir_json, tmpdir, neff_name="file.neff"):
        return _orig_compile(split_multi_waits(bir_json), tmpdir, neff_name)

    _bu.compile_bir_kernel = _compile_hook
    _b2j.compile_bir_kernel = _compile_hook


def dap(t, offset, dims):
    """DRAM AP from handle with explicit [step, count] dims (elements)."""
    return AP(tensor=t, offset=offset, ap=[list(d) for d in dims])


def build_kernel():
    _patch_tail_drain()
    nc = bass.Bass(num_devices=N_CORES)

    def par(name, shape, dtype=F32):
        return nc.declare_dram_parameter(name, list(shape), dtype, isOutput=False)

    x_ext = par("x_tokens", [NL, L], I32)
    emb_ext = par("emb", [V + 1, IN_DIM])
    edge_ext = par("edge_index", [2, E], I32)
    wihT = {d: par(f"wihT_{d}", [IN_DIM, GP]) for d in "fb"}      # gate-padded
    whhT = {d: par(f"whhT_{d}", [H, GP]) for d in "fb"}
    b_enc = {d: par(f"b_enc_{d}", [GP]) for d in "fb"}
    # Wp1.T split by input half (hf rows / hb rows) to keep matmul bases legal
    wp1T = {h: par(f"wp1T_{h}", [H, D2]) for h in "ab"}
    wp2T = {h: par(f"wp2T_{h}", [H, D2]) for h in "ab"}
    bp1 = par("bp1", [D2]); bp2 = par("bp2", [D2])
    gw = {}
    for g in ("gh", "gc"):
        gw[g] = dict(
            W1=par(f"{g}_W1", [D2, 16]), b1=par(f"{g}_b1", [16]),
            W2=par(f"{g}_W2", [16, 32]), b2=par(f"{g}_b2", [32]),
            Wfp=par(f"{g}_Wfp", [32, 64]), bfp=par(f"{g}_bfp", [64]),  # out-padded
        )
    dec2 = {d: par(f"dec2_{d}", [2, GP]) for d in "fb"}
    whhTd = {d: par(f"whhTd_{d}", [H, GP]) for d in "fb"}
    wihd_col = {d: par(f"wihd_col_{d}", [GP, 1]) for d in "fb"}
    b_dec = {d: par(f"b_dec_{d}", [GP]) for d in "fb"}
    woutT_ext = par("woutT_ext", [DR, V])     # rows 0:13 WoutT[0:13], 32:45 WoutT[13:26], 64 bout
    out_ext = nc.declare_dram_parameter("out", [NL, L, V], F32, isOutput=True)

    cc_in = nc.dram_tensor("cc_in", [D2, 2 * NL], F32)
    cc_out = nc.dram_tensor("cc_out", [N_CORES * D2, 2 * NL], F32, addr_space="Shared")
    ones_dram = nc.dram_tensor("ones_dram", [128], F32)

    with tile.TileContext(nc) as tc:
        import contextlib
        with contextlib.ExitStack() as ctx:
            const = ctx.enter_context(tc.tile_pool(name="const", bufs=1))
            work = ctx.enter_context(tc.tile_pool(name="work", bufs=3))
            encsb = ctx.enter_context(tc.tile_pool(name="encsb", bufs=3))
            decsb = ctx.enter_context(tc.tile_pool(name="decsb", bufs=3))
            stage = ctx.enter_context(tc.tile_pool(name="stage", bufs=2))
            ps_mm = ctx.enter_context(tc.tile_pool(name="ps_mm", bufs=3, space="PSUM"))
            ps_enc = ctx.enter_context(tc.tile_pool(name="ps_enc", bufs=2, space="PSUM"))
            ps_misc = ctx.enter_context(tc.tile_pool(name="ps_misc", bufs=3, space="PSUM"))

            # ============ constants & weights ============
            def load(pool, src, shape, name, dtype=F32):
                t = pool.tile(list(shape), dtype, tag=name)
                nc.sync.dma_start(out=t[:], in_=src)
                return t

            def load_col(pool, src_handle, n, name):
                t = pool.tile([n, 1], F32, tag=name)
                nc.sync.dma_start(out=t[:], in_=dap(src_handle, 0, [[1, n], [0, 1]]))
                return t

            wihT_sb = {d: load(const, wihT[d][:], [IN_DIM, GP], f"wihT{d}") for d in "fb"}
            whhT_sb = {d: load(const, whhT[d][:], [H, GP], f"whhT{d}") for d in "fb"}
            benc_sb = {d: load_col(const, b_enc[d], GP, f"benc{d}") for d in "fb"}
            wp1T_sb = {h: load(const, wp1T[h][:], [H, D2], f"wp1T{h}") for h in "ab"}
            wp2T_sb = {h: load(const, wp2T[h][:], [H, D2], f"wp2T{h}") for h in "ab"}
            bp1_sb = load_col(const, bp1, D2, "bp1")
            bp2_sb = load_col(const, bp2, D2, "bp2")
            gws = {}
            for g in ("gh", "gc"):
                gws[g] = dict(
                    W1=load(const, gw[g]["W1"][:], [D2, 16], f"{g}W1"),
                    b1=load_col(const, gw[g]["b1"], 16, f"{g}b1"),
                    W2=load(const, gw[g]["W2"][:], [16, 32], f"{g}W2"),
                    b2=load_col(const, gw[g]["b2"], 32, f"{g}b2"),
                    Wfp=load(const, gw[g]["Wfp"][:], [32, 64], f"{g}Wfp"),
                    bfp=load_col(const, gw[g]["bfp"], 64, f"{g}bfp"),
                )
            dec2_sb = {d: load(const, dec2[d][:], [2, GP], f"dec2{d}") for d in "fb"}
            whhTd_sb = {d: load(const, whhTd[d][:], [H, GP], f"whhTd{d}") for d in "fb"}
            b0p_sb = {}
            for d in "fb":
                wc = load(const, wihd_col[d][:], [GP, 1], f"wihdc{d}")
                bc = load_col(const, b_dec[d], GP, f"bdec{d}")
                b0 = const.tile([GP, 1], F32, tag=f"b0p{d}")
                nc.vector.tensor_tensor(out=b0[:], in0=bc[:], in1=wc[:], op=ALU.subtract)
                b0p_sb[d] = b0

            wout_f32 = stage.tile([128, V], F32, tag="stage")
            nc.sync.dma_start(out=wout_f32[0:DR, :], in_=woutT_ext[:])
            woutT_bf = const.tile([DR, V], BF16, tag="woutbf")
            nc.vector.tensor_copy(out=woutT_bf[:], in_=wout_f32[0:DR, :])

            ident = const.tile([128, 128], F32, tag="ident")
            make_identity(nc, ident[:])
            iota_row_i = const.tile([128, 128], I32, tag="iotarowi")
            nc.gpsimd.iota(iota_row_i[:], pattern=[[1, 128]], base=0, channel_multiplier=0)
            iota_row = const.tile([128, 128], F32, tag="iotarow")
            nc.vector.tensor_copy(out=iota_row[:], in_=iota_row_i[:])
            iota_col_i = const.tile([128, 1], I32, tag="iotacoli")
            nc.gpsimd.iota(iota_col_i[:], pattern=[[0, 1]], base=0, channel_multiplier=1)
            iota_col = const.tile([128, 1], F32, tag="iotacol")
            nc.vector.tensor_copy(out=iota_col[:], in_=iota_col_i[:])
            ones_col = const.tile([128, 1], F32, tag="onescol")
            nc.vector.memset(ones_col[:], 1.0)
            ones_row = const.tile([1, 128], F32, tag="onesrow")
            nc.vector.memset(ones_row[:], 1.0)

            # ============ tokens ============
            idx_all = const.tile([128, 16], I32, tag="idxall")
            nc.sync.dma_start(out=idx_all[:], in_=dap(x_ext, 0, [[1, 8], [L, NL], [8, 16]]))
            prev_i = const.tile([1, ROWS], I32, tag="previ")
            nc.sync.dma_start(out=prev_i[0:1, NL:ROWS], in_=dap(x_ext, 0, [[1, L - 1], [L, NL]]))
            nc.vector.memset(prev_i[0:1, 0:NL], 0)
            rhs_dec = const.tile([2, ROWS], F32, tag="rhsdec")
            nc.vector.tensor_copy(out=rhs_dec[0:1, :], in_=prev_i[0:1, :])
            nc.vector.memset(rhs_dec[0:1, 0:NL], -1.0)
            # ones row lives on partition 1: compute engines can't write there,
            # a DMA can (bounce through DRAM; outer dim repeats the 128-row)
            nc.sync.dma_start(out=ones_dram[:], in_=ones_row[0:1, :])
            nc.sync.dma_start(out=rhs_dec[1:2, :],
                              in_=dap(ones_dram, 0, [[0, ROWS // 128], [1, 128]]))

            # ============ embedding gather -> XT [64, 2048] (time-major) ============
            XT = const.tile([IN_DIM, ROWS], F32, tag="XT")
            order = []
            for i in range(8):
                order += [i, 15 - i]
            for t in order:
                gth = work.tile([128, IN_DIM], F32, tag="gather")
                nc.gpsimd.indirect_dma_start(
                    out=gth[:], out_offset=None, in_=emb_ext[:],
                    in_offset=IndirectOffsetOnAxis(ap=idx_all[:, t:t + 1], axis=0),
                )
                tp = ps_misc.tile([IN_DIM, 128], F32, tag="ps_misc")
                nc.tensor.transpose(out=tp[:], in_=gth[:], identity=ident[:])
                nc.vector.tensor_copy(out=XT[:, 128 * t:128 * (t + 1)], in_=tp[:])

            # ============ graph build (replicated) ============
            adj_ps = ps_misc.tile([128, 128], F32, tag="ps_misc")
            for k in range(NCH):
                if k < 16:
                    si = work.tile([128, 1], I32, tag="srci")
                    di = work.tile([128, 1], I32, tag="dsti")
                    nc.sync.dma_start(out=si[:], in_=dap(edge_ext, 128 * k, [[1, 128], [0, 1]]))
                    nc.sync.dma_start(out=di[:], in_=dap(edge_ext, E + 128 * k, [[1, 128], [0, 1]]))
                    sf = work.tile([128, 1], F32, tag="srcf")
                    df = work.tile([128, 1], F32, tag="dstf")
                    nc.vector.tensor_copy(out=sf[:], in_=si[:])
                    nc.vector.tensor_copy(out=df[:], in_=di[:])
                else:
                    sf = df = iota_col
                ocs = work.tile([128, 128], F32, tag="ocs")
                ocd = work.tile([128, 128], F32, tag="ocd")
                nc.vector.tensor_scalar(out=ocs[:], in0=iota_row[:], scalar1=sf[:, 0:1],
                                        scalar2=None, op0=ALU.is_equal)
                nc.vector.tensor_scalar(out=ocd[:], in0=iota_row[:], scalar1=df[:, 0:1],
                                        scalar2=None, op0=ALU.is_equal)
                nc.tensor.matmul(out=adj_ps[:], lhsT=ocs[:], rhs=ocd[:],
                                 start=(k == 0), stop=(k == NCH - 1))
            adjT = const.tile([128, 128], F32, tag="adjT")
            nc.vector.tensor_copy(out=adjT[:], in_=adj_ps[:])
            deg_ps = ps_misc.tile([1, 128], F32, tag="ps_misc")
            nc.tensor.matmul(out=deg_ps[:], lhsT=ones_col[:], rhs=adjT[:], start=True, stop=True)
            degc = work.tile([1, 128], F32, tag="degc")
            nc.vector.tensor_scalar(out=degc[:], in0=deg_ps[:], scalar1=1.0, scalar2=None,
                                    op0=ALU.max)
            sqd = work.tile([1, 128], F32, tag="sqd")
            nc.scalar.activation(out=sqd[:], in_=degc[:], func=AF.Sqrt)
            dinv_row = const.tile([1, 128], F32, tag="dinvrow")
            nc.vector.reciprocal(out=dinv_row[:], in_=sqd[:])
            dbc_ps = ps_misc.tile([128, 128], F32, tag="ps_misc")
            nc.tensor.matmul(out=dbc_ps[:], lhsT=ones_row[:], rhs=dinv_row[:], start=True, stop=True)
            dinv_bc = const.tile([128, 128], F32, tag="dinvbc")
            nc.vector.tensor_copy(out=dinv_bc[:], in_=dbc_ps[:])
            dcol_ps = ps_misc.tile([128, 1], F32, tag="ps_misc")
            nc.tensor.transpose(out=dcol_ps[:], in_=dinv_row[:], identity=ident[0:1, 0:1])
            dinv_col = const.tile([128, 1], F32, tag="dinvcol")
            nc.vector.tensor_copy(out=dinv_col[:], in_=dcol_ps[:])
            A_T = const.tile([128, 128], F32, tag="AT")
            nc.vector.tensor_scalar(out=A_T[:], in0=adjT[:], scalar1=dinv_col[:, 0:1],
                                    scalar2=None, op0=ALU.mult)
            nc.vector.tensor_tensor(out=A_T[:], in0=A_T[:], in1=dinv_bc[:], op=ALU.mult)

            # ============ encoder biLSTM (gates quadrant-padded) ============
            state = {}
            for d in "fb":
                h0 = encsb.tile([H, NL], F32, tag=f"h_{d}")
                c0 = encsb.tile([H, NL], F32, tag=f"c_{d}")
                nc.vector.memset(h0[:], 0.0)
                nc.vector.memset(c0[:], 0.0)
                state[d] = (h0, c0)

            def cell(g_ps, c_prev, bias_col, pool, nl, tagp):
                """post-matmul LSTM cell math; returns (h_new, c_new) [13, nl].

                Each gate is read from its PSUM quadrant into a base-0 SBUF
                tile (single-input ACT ops may cross quadrants; two-SBUF-input
                DVE ops may not), so all elementwise math is base-0."""
                def gate(qi, func, tag):
                    t = pool.tile([H, nl], F32, tag=f"{tag}{tagp}")
                    nc.scalar.activation(out=t[:], in_=g_ps[qi * Q:qi * Q + H, :], func=func,
                                         bias=bias_col[qi * Q:qi * Q + H, 0:1])
                    return t

                sig_i = gate(0, AF.Sigmoid, "si")
                tnh_g = gate(2, AF.Tanh, "tg")
                sig_o = gate(3, AF.Sigmoid, "so")
                t2 = pool.tile([H, nl], F32, tag=f"t2{tagp}")
                nc.vector.tensor_tensor(out=t2[:], in0=sig_i[:], in1=tnh_g[:], op=ALU.mult)
                c_new = pool.tile([H, nl], F32, tag=f"c{tagp}")
                if c_prev is not None:
                    sig_f = gate(1, AF.Sigmoid, "sf")
                    t1 = pool.tile([H, nl], F32, tag=f"t1{tagp}")
                    nc.vector.tensor_tensor(out=t1[:], in0=sig_f[:], in1=c_prev, op=ALU.mult)
                    nc.vector.tensor_tensor(out=c_new[:], in0=t1[:], in1=t2[:], op=ALU.add)
                else:
                    nc.vector.tensor_copy(out=c_new[:], in_=t2[:])
                tc_ = pool.tile([H, nl], F32, tag=f"tc{tagp}")
                nc.scalar.activation(out=tc_[:], in_=c_new[:], func=AF.Tanh)
                h_new = pool.tile([H, nl], F32, tag=f"h{tagp}")
                nc.vector.tensor_tensor(out=h_new[:], in0=sig_o[:], in1=tc_[:], op=ALU.mult)
                return h_new, c_new

            for l in range(L):
                for d, xl in (("f", l), ("b", L - 1 - l)):
                    h_prev, c_prev = state[d]
                    g = ps_enc.tile([GP, NL], F32, tag="ps_enc")
                    nc.tensor.matmul(out=g[:], lhsT=wihT_sb[d][:],
                                     rhs=XT[:, NL * xl:NL * (xl + 1)], start=True, stop=False)
                    nc.tensor.matmul(out=g[:], lhsT=whhT_sb[d][:], rhs=h_prev[:],
                                     start=False, stop=True)
                    h_new, c_new = cell(g, c_prev[:], benc_sb[d], encsb, NL, f"_{d}")
                    state[d] = (h_new, c_new)

            # ============ state projections + AllGather ============
            st_hc = work.tile([D2, 2 * NL], F32, tag="sthc")
            ph = ps_misc.tile([D2, NL], F32, tag="ps_misc")
            nc.tensor.matmul(out=ph[:], lhsT=wp1T_sb["a"][:], rhs=state["f"][0][:],
                             start=True, stop=False)
            nc.tensor.matmul(out=ph[:], lhsT=wp1T_sb["b"][:], rhs=state["b"][0][:],
                             start=False, stop=True)
            nc.vector.tensor_scalar(out=st_hc[:, 0:NL], in0=ph[:], scalar1=bp1_sb[:, 0:1],
                                    scalar2=None, op0=ALU.add)
            pc = ps_misc.tile([D2, NL], F32, tag="ps_misc")
            nc.tensor.matmul(out=pc[:], lhsT=wp2T_sb["a"][:], rhs=state["f"][1][:],
                             start=True, stop=False)
            nc.tensor.matmul(out=pc[:], lhsT=wp2T_sb["b"][:], rhs=state["b"][1][:],
                             start=False, stop=True)
            nc.vector.tensor_scalar(out=st_hc[:, NL:2 * NL], in0=pc[:], scalar1=bp2_sb[:, 0:1],
                                    scalar2=None, op0=ALU.add)
            nc.sync.dma_start(out=cc_in[:], in_=st_hc[:])
            nc.gpsimd.collective_compute(
                "AllGather", ALU.bypass,
                replica_groups=[list(range(N_CORES))],
                ins=[cc_in[:]], outs=[cc_out[:]],
            )
            shT = const.tile([D2, N], F32, tag="shT")
            scT = const.tile([D2, N], F32, tag="scT")
            nc.sync.dma_start(
                out=shT[:].rearrange("p (c n) -> p c n", c=N_CORES),
                in_=dap(cc_out, 0, [[2 * NL, D2], [D2 * 2 * NL, N_CORES], [1, NL]]),
            )
            nc.sync.dma_start(
                out=scT[:].rearrange("p (c n) -> p c n", c=N_CORES),
                in_=dap(cc_out, NL, [[2 * NL, D2], [D2 * 2 * NL, N_CORES], [1, NL]]),
            )

            # ============ GCN (replicated; output rows padded f@0, b@32) ============
            def gcn(xT_full, w):
                p1 = ps_misc.tile([N, 16], F32, tag="ps_misc")
                nc.tensor.matmul(out=p1[:], lhsT=xT_full[:], rhs=w["W1"][:], start=True, stop=True)
                xw1 = work.tile([N, 16], F32, tag="xw1")
                nc.vector.tensor_copy(out=xw1[:], in_=p1[:])
                p2 = ps_misc.tile([16, N], F32, tag="ps_misc")
                nc.tensor.matmul(out=p2[:], lhsT=xw1[:], rhs=A_T[:], start=True, stop=True)
                x1 = work.tile([16, N], F32, tag="x1")
                nc.scalar.activation(out=x1[:], in_=p2[:], func=AF.Lrelu,
                                     bias=w["b1"][:, 0:1], alpha=0.01)
                p3 = ps_misc.tile([N, 32], F32, tag="ps_misc")
                nc.tensor.matmul(out=p3[:], lhsT=x1[:], rhs=w["W2"][:], start=True, stop=True)
                xw2 = work.tile([N, 32], F32, tag="xw2")
                nc.vector.tensor_copy(out=xw2[:], in_=p3[:])
                p4 = ps_misc.tile([32, N], F32, tag="ps_misc")
                nc.tensor.matmul(out=p4[:], lhsT=xw2[:], rhs=A_T[:], start=True, stop=True)
                x2 = work.tile([32, N], F32, tag="x2")
                nc.scalar.activation(out=x2[:], in_=p4[:], func=AF.Lrelu,
                                     bias=w["b2"][:, 0:1], alpha=0.01)
                p5 = ps_misc.tile([64, N], F32, tag="ps_misc")
                nc.tensor.matmul(out=p5[:], lhsT=w["Wfp"][:], rhs=x2[:], start=True, stop=True)
                outT = work.tile([64, N], F32, tag="gout")
                nc.vector.tensor_scalar(out=outT[:], in0=p5[:], scalar1=w["bfp"][:, 0:1],
                                        scalar2=None, op0=ALU.add)
                return outT

            ghT = gcn(shT, gws["gh"])    # [64, 128]: rows 0:13 fwd, 32:45 bwd
            gcT = gcn(scT, gws["gc"])

            pid = nc.partition_id()
            col0 = pid * NL
            hT_mine = work.tile([64, NL], F32, tag="hTmine")
            cT_mine = work.tile([64, NL], F32, tag="cTmine")
            nc.vector.tensor_copy(out=hT_mine[:], in_=ghT[:, bass.ds(col0, NL)])
            nc.vector.tensor_copy(out=cT_mine[:], in_=gcT[:, bass.ds(col0, NL)])

            # ============ decoder bulk (l >= 1) ============
            decT = const.tile([DR, ROWS], BF16, tag="decT")
            nc.vector.memset(decT[0:64, :], 0.0)
            nc.vector.memset(decT[64:DR, :], 1.0)
            zero_col = const.tile([GP, 1], F32, tag="zerocol")
            nc.vector.memset(zero_col[:], 0.0)
            for d, rowbase in (("f", 0), ("b", Q)):
                for q in range(4):
                    lo = 512 * q
                    s = NL if q == 0 else 0
                    w = 512 - s
                    gd = ps_misc.tile([GP, 512], F32, tag="ps_misc")
                    nc.tensor.matmul(out=gd[:, s:512], lhsT=dec2_sb[d][:],
                                     rhs=rhs_dec[:, lo + s:lo + 512], start=True, stop=True)
                    # c0 = 0 for l>=1 so the f-gate is unused: c = sig_i*tanh(gg)
                    sig_i = decsb.tile([H, 512], F32, tag=f"si{d}")
                    nc.scalar.activation(out=sig_i[:, s:512], in_=gd[0:H, s:512],
                                         func=AF.Sigmoid, bias=zero_col[0:H, 0:1])
                    tnh_g = decsb.tile([H, 512], F32, tag=f"tg{d}")
                    nc.scalar.activation(out=tnh_g[:, s:512], in_=gd[2 * Q:2 * Q + H, s:512],
                                         func=AF.Tanh, bias=zero_col[0:H, 0:1])
                    sig_o = decsb.tile([H, 512], F32, tag=f"so{d}")
                    nc.scalar.activation(out=sig_o[:, s:512], in_=gd[3 * Q:3 * Q + H, s:512],
                                         func=AF.Sigmoid, bias=zero_col[0:H, 0:1])
                    cdec = decsb.tile([H, 512], F32, tag=f"cdec{d}")
                    nc.vector.tensor_tensor(out=cdec[:, s:512], in0=sig_i[:, s:512],
                                            in1=tnh_g[:, s:512], op=ALU.mult)
                    tcd = decsb.tile([H, 512], F32, tag=f"tcd{d}")
                    nc.scalar.activation(out=tcd[:, s:512], in_=cdec[:, s:512], func=AF.Tanh)
                    nc.vector.tensor_tensor(out=decT[rowbase:rowbase + H, lo + s:lo + 512],
                                            in0=sig_o[:, s:512], in1=tcd[:, s:512], op=ALU.mult)

            # ============ decoder head (l == 0) ============
            decH = const.tile([DR, NL], BF16, tag="decH")
            nc.vector.memset(decH[0:64, :], 0.0)
            nc.vector.memset(decH[64:DR, :], 1.0)
            hT_b = work.tile([H, NL], F32, tag="hTb")
            nc.vector.tensor_copy(out=hT_b[:], in_=hT_mine[Q:Q + H, :])
            cT_b = work.tile([H, NL], F32, tag="cTb")
            nc.vector.tensor_copy(out=cT_b[:], in_=cT_mine[Q:Q + H, :])
            for d, rowbase in (("f", 0), ("b", Q)):
                h0_rhs = hT_mine[0:H, :] if d == "f" else hT_b[:]
                c0_ap = cT_mine[0:H, :] if d == "f" else cT_b[:]
                g0 = ps_enc.tile([GP, NL], F32, tag="ps_enc")
                nc.tensor.matmul(out=g0[:], lhsT=whhTd_sb[d][:], rhs=h0_rhs,
                                 start=True, stop=True)
                h0_new, _ = cell(g0, c0_ap, b0p_sb[d], encsb, NL, f"0{d}")
                nc.vector.tensor_copy(out=decH[rowbase:rowbase + H, :], in_=h0_new[:])

            # ============ output projection + DMA out ============
            decC0 = const.tile([DR, 128], BF16, tag="decC0")
            nc.vector.tensor_copy(out=decC0[:, NL:128], in_=decT[:, NL:128])
            nc.vector.tensor_copy(out=decC0[:, 0:NL], in_=decH[:])

            def emit_chunk(m):
                st = stage.tile([128, V], F32, tag="stage")
                lhsT = decC0[:] if m == 0 else decT[:, 128 * m:128 * (m + 1)]
                for v in range(NVC):
                    ps = ps_mm.tile([128, VC], F32, tag="ps_mm")
                    nc.tensor.matmul(out=ps[:], lhsT=lhsT,
                                     rhs=woutT_bf[:, VC * v:VC * (v + 1)],
                                     start=True, stop=True)
                    nc.vector.tensor_copy(out=st[:, VC * v:VC * (v + 1)], in_=ps[:])
                nc.sync.dma_start(
                    out=dap(out_ext, 8 * m * V, [[V, 8], [L * V, NL], [1, V]]),
                    in_=st[:],
                )

            for m in range(1, NMC):
                emit_chunk(m)
            emit_chunk(0)

    return nc


# ---------------- host side ----------------
_CACHE = {}

_QMAP = np.zeros(GP, dtype=bool)
for _g in range(4):
    _QMAP[_g * Q:_g * Q + H] = True          # quadrant rows used by gates


def _pad_gates_vec(v52):
    out = np.zeros(GP, dtype=np.float32)
    out[_QMAP] = v52
    return out


def _pad_gates_cols(m):
    """[..., 52] -> [..., 128] with gate g at column g*32."""
    out = np.zeros(m.shape[:-1] + (GP,), dtype=np.float32)
    out[..., _QMAP] = m
    return out


def _get_nc():
    if "nc" not in _CACHE:
        _CACHE["nc"] = build_kernel()
    return _CACHE["nc"]


def make_in_maps(inputs):
    f32 = np.float32
    i32 = np.int32
    rep = {}
    rep["emb"] = np.ascontiguousarray(inputs["emb"], dtype=f32)
    rep["edge_index"] = np.ascontiguousarray(inputs["edge_index"], dtype=i32)
    for d in "fb":
        rep[f"wihT_{d}"] = np.ascontiguousarray(
            _pad_gates_cols(np.asarray(inputs[f"Wih_{d}_enc"], f32).T))
        rep[f"whhT_{d}"] = np.ascontiguousarray(
            _pad_gates_cols(np.asarray(inputs[f"Whh_{d}_enc"], f32).T))
        rep[f"b_enc_{d}"] = _pad_gates_vec(np.asarray(inputs[f"b_{d}_enc"], f32))
        rep[f"dec2_{d}"] = np.ascontiguousarray(_pad_gates_cols(np.stack(
            [np.asarray(inputs[f"Wih_{d}_dec"], f32)[:, 0],
             np.asarray(inputs[f"b_{d}_dec"], f32)], axis=0)))
        rep[f"whhTd_{d}"] = np.ascontiguousarray(
            _pad_gates_cols(np.asarray(inputs[f"Whh_{d}_dec"], f32).T))
        rep[f"wihd_col_{d}"] = np.ascontiguousarray(
            _pad_gates_vec(np.asarray(inputs[f"Wih_{d}_dec"], f32)[:, 0])[:, None])
        rep[f"b_dec_{d}"] = _pad_gates_vec(np.asarray(inputs[f"b_{d}_dec"], f32))
    wp1T = np.asarray(inputs["Wp1"], f32).T       # [in 26, out 26]
    wp2T = np.asarray(inputs["Wp2"], f32).T
    rep["wp1T_a"] = np.ascontiguousarray(wp1T[0:H, :])
    rep["wp1T_b"] = np.ascontiguousarray(wp1T[H:D2, :])
    rep["wp2T_a"] = np.ascontiguousarray(wp2T[0:H, :])
    rep["wp2T_b"] = np.ascontiguousarray(wp2T[H:D2, :])
    rep["bp1"] = np.ascontiguousarray(inputs["bp1"], dtype=f32)
    rep["bp2"] = np.ascontiguousarray(inputs["bp2"], dtype=f32)
    for g in ("gh", "gc"):
        for k in ("W1", "b1", "W2", "b2"):
            rep[f"{g}_{k}"] = np.ascontiguousarray(inputs[f"{g}_{k}"], dtype=f32)
        Wf = np.asarray(inputs[f"{g}_Wf"], f32)           # [32, 26]
        bf = np.asarray(inputs[f"{g}_bf"], f32)           # [26]
        Wfp = np.zeros((32, 64), f32)
        Wfp[:, 0:H] = Wf[:, 0:H]
        Wfp[:, Q:Q + H] = Wf[:, H:D2]
        bfp = np.zeros(64, f32)
        bfp[0:H] = bf[0:H]
        bfp[Q:Q + H] = bf[H:D2]
        rep[f"{g}_Wfp"] = Wfp
        rep[f"{g}_bfp"] = bfp
    woutT = np.asarray(inputs["Wout"], f32).T             # [26, 8000]
    wout_pad = np.zeros((DR, V), f32)
    wout_pad[0:H, :] = woutT[0:H, :]
    wout_pad[Q:Q + H, :] = woutT[H:D2, :]
    wout_pad[64, :] = np.asarray(inputs["bout"], f32)
    rep["woutT_ext"] = wout_pad

    x = np.ascontiguousarray(inputs["x_tokens"], dtype=i32)
    in_maps = []
    for c in range(N_CORES):
        m = dict(rep)
        m["x_tokens"] = np.ascontiguousarray(x[NL * c:NL * (c + 1)])
        in_maps.append(m)
    return in_maps


def kernel(**inputs):
    nc = _get_nc()
    in_maps = make_in_maps(inputs)
    res = run_bass_kernel_spmd(nc, in_maps, core_ids=list(range(N_CORES)), trace=False)
    out = np.concatenate([res.results[c]["out"] for c in range(N_CORES)], axis=0)
    return out.astype(np.float32)
